# revision 13
# baseline (speedup 1.0000x reference)
"""Trainium2 Bass kernel for the nn_Detect head (3-level YOLO-style decode).

Strategy: data-parallel over batch (8 images -> 8 NeuronCores). Per core:
  - 3x3 convs as 9-tap shifted matmuls (bf16) accumulating in PSUM,
    activations kept [cin, px] in SBUF with x/y zero-padding done on host.
  - 1x1 head convs computed as act_chunk.T @ W so the matmul itself emits
    [px, ch] tiles (transpose for free), 128 pixels on partitions.
  - decode (sigmoid/grid/anchor/dims) as wide-partition vector/scalar ops.
  - orientation L2-normalize deferred to one Sqrt + reciprocal pass at the
    end to avoid ACT table-set thrashing with Sigmoid.
Inputs are cast/packed to bf16 on host; accumulation is fp32.
"""

import numpy as np
import ml_dtypes

bf16 = ml_dtypes.bfloat16

BS = 8
NCORES = 8
NO2D = 8
NO3D = 9
NOUT = 17
NROWS = 25200

# (C, H, W, stride, Rb_rows, slab_rows_list)
LEVELS = [
    (256, 80, 80, 8.0, 4, [8] * 10),
    (512, 40, 40, 16.0, 8, [16, 16, 8]),
    (1024, 20, 20, 32.0, 20, [20]),
]
ANCHORS = np.array(
    [
        [[10, 13], [16, 30], [33, 23]],
        [[30, 61], [62, 45], [59, 119]],
        [[116, 90], [156, 198], [373, 326]],
    ],
    np.float32,
)

_S = [H * W for (_, H, W, _, _, _) in LEVELS]          # 6400, 1600, 400
_NCH = [(s + 127) // 128 for s in _S]                  # 50, 13, 4
_ROW0 = [0, 3 * _S[0], 3 * _S[0] + 3 * _S[1]]          # level row offsets
_NBOFF = []
_off = 0
for _l in range(3):
    for _a in range(3):
        _NBOFF.append(_off)
        _off += _NCH[_l]
_NBTOT = _off                                           # 201

FP8 = False          # 3x3 convs in fp8 e4m3 with DoubleRow (2 k-chunks/matmul)
WSCALE = 64.0        # host premultiplier for fp8 3x3 weights (std 0.01 is subnormal)
_W2P = [88, 48, 24]  # x-padded width per level: (H+2)*W2P % 16 == 0 for k-pair APs

_PROGRAM_CACHE = {}


def _groups(nchunks, g=16):
    out = []
    k0 = 0
    while k0 < nchunks:
        gn = min(g, nchunks - k0)
        out.append((k0, gn))
        k0 += gn
    return out


def _build_program(bias_flags):
    import concourse.mybir as mybir
    import concourse.tile as tile
    from concourse import bacc

    (has_ba, has_bb) = bias_flags  # tuples of 6 bools: (l, branch) order

    nc = bacc.Bacc(None)
    f32 = mybir.dt.float32
    b16 = mybir.dt.float16
    cdt = mybir.dt.float8e4 if FP8 else b16   # conv (3x3) storage dtype

    fps = []
    was = []
    wbs = []
    grids = []
    for l, (C, H, W, _, _, _) in enumerate(LEVELS):
        Q = C // 128
        W2 = _W2P[l] if FP8 else W + 2
        fps.append(
            nc.declare_dram_parameter(f"f{l}p", [128, Q, H + 2, W2], cdt, isOutput=False)
        )
        was.append(
            [
                nc.declare_dram_parameter(f"wa{b}{l}", [Q, 128, Q, 9, 128], cdt, isOutput=False)
                for b in (2, 3)
            ]
        )
        wbs.append(
            [
                nc.declare_dram_parameter(
                    f"wb{b}{l}", [128, Q, NO2D * 3 if b == 2 else NO3D * 3], b16, isOutput=False
                )
                for b in (2, 3)
            ]
        )
        grids.append(
            nc.declare_dram_parameter(f"grid{l}", [128, _NCH[l], 2], f32, isOutput=False)
        )
    anch = nc.declare_dram_parameter("anch", [128, 3, 3, 2], f32, isOutput=False)
    bas = {}
    bbs = {}
    for l, (C, H, W, _, _, _) in enumerate(LEVELS):
        Q = C // 128
        for bi, b in enumerate((2, 3)):
            if has_ba[l * 2 + bi]:
                bas[(l, b)] = nc.declare_dram_parameter(f"ba{b}{l}", [128, Q], f32, isOutput=False)
            if has_bb[l * 2 + bi]:
                nchn = NO2D * 3 if b == 2 else NO3D * 3
                bbs[(l, b)] = nc.declare_dram_parameter(f"bb{b}{l}", [128, nchn // 3], f32, isOutput=False)
    out = nc.declare_dram_parameter("out", [NROWS, NOUT], f32, isOutput=True)

    with tile.TileContext(nc) as tc:
        from contextlib import ExitStack

        with ExitStack() as ctx:
            cpool = ctx.enter_context(tc.tile_pool(name="consts", bufs=1))
            spool = ctx.enter_context(tc.tile_pool(name="stage", bufs=1))
            ipool = ctx.enter_context(tc.tile_pool(name="inbuf", bufs=1))
            a2pool = ctx.enter_context(tc.tile_pool(name="act2", bufs=1))
            a3pool = ctx.enter_context(tc.tile_pool(name="act3", bufs=1))
            wpool = ctx.enter_context(tc.tile_pool(name="w3", bufs=2))
            p3pool = ctx.enter_context(tc.tile_pool(name="psum3", bufs=4, space="PSUM"))
            h2pool = ctx.enter_context(tc.tile_pool(name="h2p", bufs=2, space="PSUM"))
            h3pool = ctx.enter_context(tc.tile_pool(name="h3p", bufs=2, space="PSUM"))
            scpool = ctx.enter_context(tc.tile_pool(name="scratch", bufs=2))

            # ---- constants ----
            gts = []
            wbt = []
            for l, (C, H, W, _, _, _) in enumerate(LEVELS):
                Q = C // 128
                gt = cpool.tile([128, _NCH[l], 2], f32, tag=f"grid{l}")
                nc.sync.dma_start(gt[:], grids[l][:])
                gts.append(gt)
                w2t = cpool.tile([128, Q, NO2D * 3], b16, tag=f"wb2{l}")
                nc.sync.dma_start(w2t[:], wbs[l][0][:])
                w3t = cpool.tile([128, Q, NO3D * 3], b16, tag=f"wb3{l}")
                nc.sync.dma_start(w3t[:], wbs[l][1][:])
                wbt.append((w2t, w3t))
            ancht = cpool.tile([128, 3, 3, 2], f32)
            nc.sync.dma_start(ancht[:], anch[:])
            epst = cpool.tile([128, 1], f32)
            nc.vector.memset(epst[:], 1e-24)
            bat = {}
            bbt = {}
            for (l, b), p in bas.items():
                t = cpool.tile(list(p.shape), f32, tag=f"ba{b}{l}")
                nc.sync.dma_start(t[:], p[:])
                bat[(l, b)] = t
            for (l, b), p in bbs.items():
                t = cpool.tile(list(p.shape), f32, tag=f"bb{b}{l}")
                nc.sync.dma_start(t[:], p[:])
                bbt[(l, b)] = t

            # ---- staging + norm buffers (persist to end) ----
            st = [
                [
                    spool.tile([128, _NCH[l], NOUT], f32, tag=f"st{l}{a}", name=f"st{l}{a}")
                    for a in range(3)
                ]
                for l in range(3)
            ]
            nb = spool.tile([128, _NBTOT, 2], f32)

            # prefetch all levels' features up front (per-chunk DMAs so the
            # first matmuls only wait on the chunk they read; parity slots
            # let level l+1 load while level l computes)
            inb = [None, None, None]
            for l, (C, H, W, _, _, _) in enumerate(LEVELS):
                Q = C // 128
                W2 = _W2P[l] if FP8 else W + 2
                it = ipool.tile([128, Q, H + 2, W2], cdt, tag=f"inb{l % 2}", name=f"inb{l}")
                for q in range(Q):
                    nc.sync.dma_start(it[:, q], fps[l][:, q])
                inb[l] = it

            copy_ctr = 0
            for l, (C, H, W, stride, Rb, slab_rows) in enumerate(LEVELS):
                Q = C // 128
                S = H * W
                N = Rb * W

                acts = []
                for bi, b in enumerate((2, 3)):
                    pool = a2pool if b == 2 else a3pool
                    act = pool.tile([128, Q, S], b16, tag=f"act{b}_{l % 2}", name=f"act{b}_{l}")
                    acts.append(act)
                    for j in range(Q):
                        wt = wpool.tile([128, Q, 9, 128], cdt, tag="w3t")
                        for q in range(Q):
                            nc.sync.dma_start(wt[:, q], was[l][bi][j, :, q])
                        r0 = 0
                        for rows in slab_rows:
                            nblk = rows // Rb
                            blks = [
                                p3pool.tile([128, 512], f32, tag="blk", name="blk")
                                for _ in range(nblk)
                            ]
                            if FP8:
                                import concourse.mybir as _mb
                                QP = Q // 2
                                for qp in range(QP):
                                    for t in range(9):
                                        ty, tx = divmod(t, 3)
                                        for bl in range(nblk):
                                            br0 = r0 + bl * Rb
                                            rhs = inb[l][
                                                :, 2 * qp : 2 * qp + 2,
                                                br0 + ty : br0 + ty + Rb, tx : tx + W,
                                            ]
                                            nc.tensor.matmul(
                                                blks[bl][:, :N],
                                                wt[:, 2 * qp : 2 * qp + 2, t, :],
                                                rhs,
                                                start=(t == 0 and qp == 0),
                                                stop=(t == 8 and qp == QP - 1),
                                                perf_mode=_mb.MatmulPerfMode.DoubleRow,
                                            )
                            else:
                                for q in range(Q):
                                    for t in range(9):
                                        ty, tx = divmod(t, 3)
                                        for bl in range(nblk):
                                            br0 = r0 + bl * Rb
                                            rhs = inb[l][:, q, br0 + ty : br0 + ty + Rb, tx : tx + W]
                                            nc.tensor.matmul(
                                                blks[bl][:, :N],
                                                wt[:, q, t, :],
                                                rhs,
                                                start=(t == 0 and q == 0),
                                                stop=(t == 8 and q == Q - 1),
                                            )
                            for bl in range(nblk):
                                px0 = (r0 + bl * Rb) * W
                                dst = act[:, j, px0 : px0 + N]
                                batile = bat.get((l, b))
                                rescale = 1.0 / WSCALE if FP8 else 1.0
                                if batile is not None:
                                    nc.vector.tensor_scalar(
                                        dst, blks[bl][:, :N], rescale, batile[:, j : j + 1],
                                        mybir.AluOpType.mult, mybir.AluOpType.add,
                                    )
                                elif not FP8 and copy_ctr % 2 == 0:
                                    nc.vector.tensor_copy(dst, blks[bl][:, :N])
                                elif not FP8:
                                    nc.scalar.copy(dst, blks[bl][:, :N])
                                elif copy_ctr % 2 == 0:
                                    nc.vector.tensor_scalar_mul(dst, blks[bl][:, :N], rescale)
                                else:
                                    nc.scalar.mul(dst, blks[bl][:, :N], rescale)
                                copy_ctr += 1
                            r0 += rows

                act2, act3 = acts
                w2t, w3t = wbt[l]
                sig = mybir.ActivationFunctionType.Sigmoid
                mult = mybir.AluOpType.mult
                add = mybir.AluOpType.add

                for (k0, gn) in _groups(_NCH[l]):
                    h2p = h2pool.tile([128, 16, NO2D * 3], f32, tag="h2p")
                    h3p = h3pool.tile([128, 16, NO3D * 3], f32, tag="h3p")
                    for gi in range(gn):
                        px0 = (k0 + gi) * 128
                        M = min(128, S - px0)
                        for q in range(Q):
                            nc.tensor.matmul(
                                h2p[:M, gi, :],
                                act2[:, q, px0 : px0 + M],
                                w2t[:, q, :],
                                start=(q == 0),
                                stop=(q == Q - 1),
                            )
                        for q in range(Q):
                            nc.tensor.matmul(
                                h3p[:M, gi, :],
                                act3[:, q, px0 : px0 + M],
                                w3t[:, q, :],
                                start=(q == 0),
                                stop=(q == Q - 1),
                            )
                    bb2 = bbt.get((l, 2))
                    if bb2 is not None:
                        nc.vector.tensor_tensor(
                            h2p[:, :gn, :], h2p[:, :gn, :],
                            bb2[:, None, :].to_broadcast([128, gn, NO2D * 3]), add,
                        )
                    bb3 = bbt.get((l, 3))
                    if bb3 is not None:
                        nc.vector.tensor_tensor(
                            h3p[:, :gn, :], h3p[:, :gn, :],
                            bb3[:, None, :].to_broadcast([128, gn, NO3D * 3]), add,
                        )
                    for a in range(3):
                        sta = st[l][a]
                        cols = sta[:, k0 : k0 + gn, :]
                        # h2: sigmoid all 8 channels
                        nc.scalar.activation(cols[:, :, 0:NO2D], h2p[:, :gn, NO2D * a : NO2D * (a + 1)], sig)
                        # xy: sig*2s + (grid-0.5)*s
                        nc.vector.tensor_scalar_mul(cols[:, :, 0:2], cols[:, :, 0:2], 2.0 * stride)
                        nc.vector.tensor_tensor(cols[:, :, 0:2], cols[:, :, 0:2], gts[l][:, k0 : k0 + gn, :], add)
                        # wh: (2 sig)^2 A = sig^2 * 4A
                        nc.vector.tensor_tensor(cols[:, :, 2:4], cols[:, :, 2:4], cols[:, :, 2:4], mult)
                        nc.vector.tensor_tensor(
                            cols[:, :, 2:4], cols[:, :, 2:4],
                            ancht[:, l, a, :][:, None, :].to_broadcast([128, gn, 2]), mult,
                        )
                        # h3 bins+orient raw copy
                        nc.vector.tensor_copy(cols[:, :, 8:14], h3p[:, :gn, NO3D * a : NO3D * a + 6])
                        # orient norm^2 -> norm buffer
                        sqt = scpool.tile([128, 16, 4], f32, tag="sqt")
                        nc.vector.tensor_tensor(sqt[:, :gn, :], cols[:, :, 10:14], cols[:, :, 10:14], mult)
                        sq4 = sqt[:, :gn, :].rearrange("p g (j t) -> p g j t", t=2)
                        noff = _NBOFF[l * 3 + a]
                        nc.vector.tensor_tensor(
                            nb[:, noff + k0 : noff + k0 + gn, :], sq4[:, :, :, 0], sq4[:, :, :, 1], add
                        )
                        # dims: sigmoid * 2 - 1
                        nc.scalar.activation(cols[:, :, 14:17], h3p[:, :gn, NO3D * a + 6 : NO3D * a + 9], sig)
                        nc.vector.tensor_scalar(cols[:, :, 14:17], cols[:, :, 14:17], 2.0, -1.0, mult, add)

                # per-level orientation normalize + output DMA, so the store
                # tail overlaps the next level's compute
                loff = _NBOFF[l * 3]
                nbl = nb[:, loff : loff + 3 * _NCH[l], :]
                nc.scalar.activation(nbl, nbl, mybir.ActivationFunctionType.Sqrt, bias=epst[:])
                nc.vector.reciprocal(nbl, nbl)
                kfull, rem = divmod(S, 128)
                for a in range(3):
                    noff = _NBOFF[l * 3 + a]
                    ori = st[l][a][:, :, 10:14].rearrange("p k (j t) -> p k j t", t=2)
                    rinv = nb[:, noff : noff + _NCH[l], :][:, :, :, None].to_broadcast(
                        [128, _NCH[l], 2, 2]
                    )
                    nc.vector.tensor_tensor(ori, ori, rinv, mult)
                    row0 = _ROW0[l] + a * S
                    half = (kfull + 1) // 2
                    for c0, c1 in ((0, half), (half, kfull)):
                        if c1 > c0:
                            nc.sync.dma_start(
                                out[row0 + c0 * 128 : row0 + c1 * 128, :].rearrange(
                                    "(k p) c -> p k c", p=128
                                ),
                                st[l][a][:, c0:c1, :],
                            )
                    if rem:
                        nc.sync.dma_start(
                            out[row0 + kfull * 128 : row0 + S, :],
                            st[l][a][:rem, kfull, :],
                        )

    nc.finalize()
    return nc


def _pack_inputs(inputs):
    """Host-side packing: pad activations, transpose+cast weights to fp16."""
    shared = {}
    percore = [dict() for _ in range(BS)]
    for l, (C, H, W, stride, _, _) in enumerate(LEVELS):
        Q = C // 128
        S = H * W
        f = np.asarray(inputs[f"f{l}"])
        cdt_np = ml_dtypes.float8_e4m3 if FP8 else np.float16
        W2 = _W2P[l] if FP8 else W + 2
        fpad = np.zeros((BS, C, H + 2, W2), cdt_np)
        fpad[:, :, 1 : H + 1, 1 : W + 1] = f.astype(cdt_np)
        fp = np.ascontiguousarray(
            fpad.reshape(BS, Q, 128, H + 2, W2).transpose(0, 2, 1, 3, 4)
        )
        for bcore in range(BS):
            percore[bcore][f"f{l}p"] = fp[bcore]

        for b, wkey in ((2, f"w2a{l}"), (3, f"w3a{l}")):
            w = np.asarray(inputs[wkey])  # [C, C, 3, 3]
            if FP8:
                w = w * np.float32(WSCALE)
            w6 = w.reshape(Q, 128, Q, 128, 3, 3)  # [j, c, q, p, ky, kx]
            shared[f"wa{b}{l}"] = np.ascontiguousarray(
                w6.transpose(0, 3, 2, 4, 5, 1).reshape(Q, 128, Q, 9, 128).astype(cdt_np)
            )
        for b, wkey, nch in ((2, f"w2b{l}", NO2D * 3), (3, f"w3b{l}", NO3D * 3)):
            w = np.asarray(inputs[wkey])[:, :, 0, 0]  # [nch, C]
            shared[f"wb{b}{l}"] = np.ascontiguousarray(
                w.T.reshape(Q, 128, nch).transpose(1, 0, 2).astype(np.float16)
            )

        px = np.arange(_NCH[l] * 128, dtype=np.float32)
        gx = np.where(px < S, px % W, 0.0).astype(np.float32)
        gy = np.where(px < S, px // W, 0.0).astype(np.float32)
        g = np.stack([(gx - 0.5) * stride, (gy - 0.5) * stride], -1)
        shared[f"grid{l}"] = np.ascontiguousarray(
            g.reshape(_NCH[l], 128, 2).transpose(1, 0, 2)
        )

    shared["anch"] = np.ascontiguousarray(
        np.broadcast_to(4.0 * ANCHORS[None], (128, 3, 3, 2)).astype(np.float32)
    )

    has_ba = []
    has_bb = []
    for l, (C, _, _, _, _, _) in enumerate(LEVELS):
        Q = C // 128
        for b, akey, bkey in ((2, f"b2a{l}", f"b2b{l}"), (3, f"b3a{l}", f"b3b{l}")):
            ba = np.asarray(inputs[akey])
            bb = np.asarray(inputs[bkey])
            nz_a = bool(np.any(ba != 0))
            nz_b = bool(np.any(bb != 0))
            has_ba.append(nz_a)
            has_bb.append(nz_b)
            if nz_a:
                shared[f"ba{b}{l}"] = np.ascontiguousarray(
                    ba.reshape(Q, 128).T.astype(np.float32)
                )
            if nz_b:
                shared[f"bb{b}{l}"] = np.ascontiguousarray(
                    np.broadcast_to(bb[None], (128, bb.shape[0])).astype(np.float32)
                )

    in_maps = []
    for bcore in range(BS):
        m = dict(shared)
        m.update(percore[bcore])
        in_maps.append(m)
    return in_maps, (tuple(has_ba), tuple(has_bb))


def _get_program(bias_flags):
    if bias_flags not in _PROGRAM_CACHE:
        _PROGRAM_CACHE[bias_flags] = _build_program(bias_flags)
    return _PROGRAM_CACHE[bias_flags]


def _run(inputs, trace=False):
    from concourse.bass_utils import run_bass_kernel_spmd

    in_maps, bias_flags = _pack_inputs(inputs)
    nc = _get_program(bias_flags)
    res = run_bass_kernel_spmd(
        nc, in_maps, core_ids=list(range(NCORES)), trace=trace
    )
    outp = np.stack([res.results[i]["out"] for i in range(NCORES)]).astype(np.float32)
    return outp, res


def kernel(**inputs) -> np.ndarray:
    outp, _ = _run(inputs, trace=False)
    return outp
